# revision 6
# baseline (speedup 1.0000x reference)
"""Trainium2 Bass kernel for nn_NeuralMemory (scatter_memory).

Computation (see reference): per-token MLP loss gradients w.r.t. W0..W3 are
rank-1 outer products gv_l[t] (x) av_l[t]; the output cum_grads is their
cumulative sum over the sequence, next_mem = W + cum[-1], plus per-token
losses.

Sharding: 8 cores, each owns (batch b = core//4, quarter q = core%4) a
256-token contiguous slice.  Cores recompute the fwd/bwd vectors of all
*preceding* tokens in their batch (zero-padded to a fixed 768) to build the
cross-chunk carry locally — fully SPMD, no collectives.

Per-core algorithm (all fp32):
  - fwd/bwd per 128-token chunk in "column" layout (features on partitions):
    kv projection, 4-layer MLP with silu, backprop to get gv_l / av_l, then
    PE-transpose to "row" layout (tokens on partitions).
  - carry: accumulated K=128 chunk-sum matmuls in one PSUM bank
    (4 layer slots of 128 fp32).
  - scan (the big part): for each token t of an own chunk, ONE matmul
    computes the full within-chunk prefix Gr[0:t+1].T @ Ar[0:t+1] into its
    own PSUM slot (K=t+1, N=128); the chunk carry is pre-injected into each
    slot by an identity matmul (N=512 per bank).  Whole-bank (128,512)
    copies evict 4 tokens at a time, alternating Vector/Scalar engines, into
    16-token SBUF staging tiles that are DMAed to DRAM (1 MiB per DMA,
    512 B descriptors).
"""

import numpy as np
from contextlib import ExitStack

import concourse.bass as bass
import concourse.bacc as bacc
import concourse.mybir as mybir
import concourse.tile as tile
from concourse.masks import make_identity

F32 = mybir.dt.float32
F16 = mybir.dt.float16
AF = mybir.ActivationFunctionType
OP = mybir.AluOpType

D = 128
B, N = 2, 1024
NCORES = 8
QUARTERS = 4
CHUNK = 128
TOK_OWN = N // QUARTERS              # 256
OWN_CHUNKS = TOK_OWN // CHUNK        # 2
PREC_CHUNKS = (QUARTERS - 1) * TOK_OWN // CHUNK  # 6
GB = 8                               # tokens per psum scan group (2 banks)
SG = 16                              # tokens per staging tile / DMA

# Use the hardware Silu / Derivative_silu activation LUTs (not modeled in
# CoreSim).  False = decompose via Sigmoid (validated in sim).
USE_LUT_SILU = True

# dtype of the scan operands (Gr/Ar rows, identity, carry replicas).  fp16
# gives single-pass PE matmuls + fast weight load (fp32 is hi/lo split = 2x).
SCAN_DT = F16


def build_program(own_chunks=OWN_CHUNKS, prec_chunks=PREC_CHUNKS):
    nc = bacc.Bacc("TRN2")
    n_own = own_chunks * CHUNK
    n_prec = prec_chunks * CHUNK

    seq_own = nc.dram_tensor("seq_own", [n_own, D], F32, kind="ExternalInput")
    seq_prec = nc.dram_tensor("seq_prec", [n_prec, D], F32, kind="ExternalInput")
    wkv_d = nc.dram_tensor("Wkv", [2 * D, D], F32, kind="ExternalInput")
    w_d = [nc.dram_tensor(f"W{l}", [D, D], F32, kind="ExternalInput") for l in range(4)]
    cum_out = nc.dram_tensor("cum_part", [4, n_own, D, D], F32, kind="ExternalOutput")
    loss_out = nc.dram_tensor("loss_part", [1, n_own], F32, kind="ExternalOutput")
    nm_out = nc.dram_tensor("nm_part", [4, D, D], F32, kind="ExternalOutput")

    with ExitStack() as ctx:
        tc = ctx.enter_context(tile.TileContext(nc))
        const = ctx.enter_context(tc.tile_pool(name="const", bufs=1))
        work = ctx.enter_context(tc.tile_pool(name="work", bufs=2))
        rows_own = ctx.enter_context(tc.tile_pool(name="rows_own", bufs=1))
        rows_prec = ctx.enter_context(tc.tile_pool(name="rows_prec", bufs=2))
        cq_pool = ctx.enter_context(tc.tile_pool(name="cq", bufs=2))
        stage_pool = ctx.enter_context(tc.tile_pool(name="stage", bufs=3))
        psum_work = ctx.enter_context(tc.tile_pool(name="psum_work", bufs=2, space="PSUM"))
        psum_carry = ctx.enter_context(tc.tile_pool(name="psum_carry", bufs=1, space="PSUM"))
        psum_scan = ctx.enter_context(tc.tile_pool(name="psum_scan", bufs=2, space="PSUM"))

        # ---- constants / weights ----
        ident = const.tile([D, D], F32, tag="ident")
        make_identity(nc, ident[:])
        ones = const.tile([D, 1], F32, tag="ones")
        nc.vector.memset(ones[:], 1.0)

        w_raw = []   # Wk, Wv, W0..W3 as stored (row i on partition i)
        names = ["wk", "wv", "w0", "w1", "w2", "w3"]
        srcs = [wkv_d[0:D, :], wkv_d[D : 2 * D, :]] + [w[:] for w in w_d]
        for name, src in zip(names, srcs):
            t = const.tile([D, D], F32, tag=name)
            nc.gpsimd.dma_start(t[:], src)
            w_raw.append(t)
        w_t = []     # transposed copies (column k on partition k)
        for name, t in zip(names, w_raw):
            tp = psum_work.tile([D, D], F32, tag="work")
            nc.tensor.transpose(tp[:], t[:], ident[:])
            tt = const.tile([D, D], F32, tag=name + "T")
            nc.vector.tensor_copy(tt[:], tp[:])
            w_t.append(tt)
        wkT, wvT = w_t[0], w_t[1]
        wT = w_t[2:]          # fwd lhsT (W_l^T)
        wraw = w_raw[2:]      # bwd lhsT (W_l)

        loss_sb = const.tile([1, n_own], F32, tag="loss")

        # ---- fwd/bwd for one 128-token chunk ----
        def fwd_bwd(src_ap, rowpool, tag_sfx, loss_slice=None):
            x_r = work.tile([CHUNK, D], F32, tag="x_r")
            nc.gpsimd.dma_start(x_r[:], src_ap)
            xp = psum_work.tile([D, CHUNK], F32, tag="work")
            nc.tensor.transpose(xp[:], x_r[:], ident[:])
            x_c = work.tile([D, CHUNK], F32, tag="x_c")
            nc.scalar.copy(x_c[:], xp[:])

            kp = psum_work.tile([D, CHUNK], F32, tag="work")
            nc.tensor.matmul(kp[:], wkT[:], x_c[:])
            k_s = work.tile([D, CHUNK], F32, tag="k_s")
            nc.scalar.copy(k_s[:], kp[:])
            vp = psum_work.tile([D, CHUNK], F32, tag="work")
            nc.tensor.matmul(vp[:], wvT[:], x_c[:])
            v_s = work.tile([D, CHUNK], F32, tag="v_s")
            nc.scalar.mul(v_s[:], vp[:], 2.0 / D)

            acts = [k_s]
            dsil = []
            cur = k_s
            for l in range(3):
                zp = psum_work.tile([D, CHUNK], F32, tag="work")
                nc.tensor.matmul(zp[:], wT[l][:], cur[:])
                a = work.tile([D, CHUNK], F32, tag=f"a{l}")
                d = work.tile([D, CHUNK], F32, tag=f"d{l}")
                if USE_LUT_SILU:
                    nc.scalar.activation(a[:], zp[:], AF.Silu)
                    nc.scalar.activation(d[:], zp[:], AF.Derivative_silu)
                else:
                    s = work.tile([D, CHUNK], F32, tag=f"s{l}")
                    nc.scalar.activation(s[:], zp[:], AF.Sigmoid)
                    nc.vector.tensor_tensor(a[:], zp[:], s[:], OP.mult)
                    m = work.tile([D, CHUNK], F32, tag=f"m{l}")
                    nc.vector.tensor_tensor(m[:], a[:], s[:], OP.mult)
                    nc.vector.tensor_tensor(m[:], a[:], m[:], OP.subtract)
                    nc.vector.tensor_tensor(d[:], s[:], m[:], OP.add)
                acts.append(a)
                dsil.append(d)
                cur = a
            z4p = psum_work.tile([D, CHUNK], F32, tag="work")
            nc.tensor.matmul(z4p[:], wT[3][:], cur[:])
            e = work.tile([D, CHUNK], F32, tag="e")
            nc.vector.scalar_tensor_tensor(e[:], z4p[:], 2.0 / D, v_s[:], OP.mult, OP.subtract)

            if loss_slice is not None:
                sq = work.tile([D, CHUNK], F32, tag="sq")
                nc.scalar.activation(sq[:], e[:], AF.Square)
                lp = psum_work.tile([1, CHUNK], F32, tag="work")
                nc.tensor.matmul(lp[:], ones[:], sq[:])
                nc.scalar.mul(loss_slice, lp[:], D / 4.0)

            gvs = [None, None, None, e]
            for l in (3, 2, 1):
                up = psum_work.tile([D, CHUNK], F32, tag="work")
                nc.tensor.matmul(up[:], wraw[l][:], gvs[l][:])
                g = work.tile([D, CHUNK], F32, tag=f"gv{l - 1}")
                nc.vector.tensor_tensor(g[:], up[:], dsil[l - 1][:], OP.mult)
                gvs[l - 1] = g

            Gr, Ar = [], []
            for l in range(4):
                gp = psum_work.tile([CHUNK, D], F32, tag="work")
                nc.tensor.transpose(gp[:], gvs[l][:], ident[:])
                gr = rowpool.tile([CHUNK, D], SCAN_DT, tag=f"gr{l}{tag_sfx}")
                nc.scalar.copy(gr[:], gp[:])
                ap_ = psum_work.tile([CHUNK, D], F32, tag="work")
                nc.tensor.transpose(ap_[:], acts[l][:], ident[:])
                ar = rowpool.tile([CHUNK, D], SCAN_DT, tag=f"ar{l}{tag_sfx}")
                nc.scalar.copy(ar[:], ap_[:])
                Gr.append(gr)
                Ar.append(ar)
            return Gr, Ar

        # ---- phase 1: preceding chunks -> carry (4 layer slots in 1 bank) ----
        carry_p = psum_carry.tile([D, 4 * D], F32, tag="carry")
        for pc in range(prec_chunks):
            GrP, ArP = fwd_bwd(seq_prec[pc * CHUNK : (pc + 1) * CHUNK, :], rows_prec, "p")
            for l in range(4):
                nc.tensor.matmul(
                    carry_p[:, l * D : (l + 1) * D],
                    GrP[l][:],
                    ArP[l][:],
                    start=(pc == 0 and l == 0),
                    stop=(pc == prec_chunks - 1 and l == 3),
                )
        carry_sb = const.tile([D, 4 * D], F32, tag="carry_sb")
        nc.vector.tensor_copy(carry_sb[:], carry_p[:])

        # ---- phase 2: own chunks fwd/bwd (+losses), rows kept resident ----
        own_rows = []
        for oc in range(own_chunks):
            Gr, Ar = fwd_bwd(
                seq_own[oc * CHUNK : (oc + 1) * CHUNK, :],
                rows_own,
                f"o{oc}",
                loss_slice=loss_sb[0:1, oc * CHUNK : (oc + 1) * CHUNK],
            )
            own_rows.append((Gr, Ar))

        # ---- phase 2b: chunk-0 sums -> chunk-1 carry (PE, decoupled) ----
        carry_srcs = [carry_sb]
        if own_chunks > 1:
            s0_p = psum_carry.tile([D, 4 * D], F32, tag="carry")
            Gr0, Ar0 = own_rows[0]
            for l in range(4):
                nc.tensor.matmul(
                    s0_p[:, l * D : (l + 1) * D], Gr0[l][:], Ar0[l][:],
                    start=(l == 0), stop=(l == 3),
                )
            comb_sb = const.tile([D, 4 * D], F32, tag="comb_sb")
            nc.vector.tensor_tensor(comb_sb[:], s0_p[:], carry_sb[:], OP.add)
            carry_srcs.append(comb_sb)

        # ---- phase 3: scan ----
        last_stage = [None] * 4
        for oc in range(own_chunks):
            # per-layer fp32 carry, replicated 4x (one bank) for fused
            # carry-add during DVE eviction
            cqs = []
            for l in range(4):
                cq = cq_pool.tile([D, 4 * D], F32, tag=f"cq{l}")
                src = carry_srcs[oc][:, l * D : (l + 1) * D]
                for r in range(4):
                    nc.gpsimd.tensor_copy(cq[:, r * D : (r + 1) * D], src)
                cqs.append(cq)

            Gr, Ar = own_rows[oc]
            n_base = oc * CHUNK
            for l in range(4):
                for sg0 in range(0, CHUNK, SG):
                    stage = stage_pool.tile([D, SG * D], F32, tag=f"stage{l}")
                    for g0 in range(sg0, sg0 + SG, GB):
                        scan = psum_scan.tile([D, GB * D], F32, tag="scan")
                        for j in range(GB):
                            t = g0 + j
                            nc.tensor.matmul(
                                scan[:, j * D : (j + 1) * D],
                                Gr[l][0 : t + 1, :],
                                Ar[l][0 : t + 1, :],
                                start=(j % 4 == 0),
                                stop=(j % 4 == 3),
                            )
                        for bank in range(GB // 4):
                            ssl = slice((g0 - sg0 + bank * 4) * D, (g0 - sg0 + bank * 4 + 4) * D)
                            psl = slice(bank * 4 * D, (bank + 1) * 4 * D)
                            nc.vector.tensor_tensor(
                                stage[:, ssl], scan[:, psl], cqs[l][:], OP.add
                            )
                    dma_eng = nc.sync if (l + sg0 // SG) % 2 == 0 else nc.scalar
                    dma_eng.dma_start(
                        cum_out[l, n_base + sg0 : n_base + sg0 + SG].rearrange("n i j -> i n j"),
                        stage[:].rearrange("p (n j) -> p n j", n=SG),
                    )
                    if sg0 + SG == CHUNK:
                        last_stage[l] = stage

        # ---- phase 4: next_mem + losses out ----
        for l in range(4):
            nm = work.tile([D, D], F32, tag=f"nm{l}")
            nc.vector.tensor_tensor(
                nm[:], last_stage[l][:, (SG - 1) * D : SG * D], wraw[l][:], OP.add
            )
            nc.sync.dma_start(nm_out[l], nm[:])
        nc.sync.dma_start(loss_out[:], loss_sb[:])

    nc.finalize()
    return nc


_PROG = None


def _get_program():
    global _PROG
    if _PROG is None:
        _PROG = build_program()
    return _PROG


def make_in_maps(seq, Wkv, W0, W1, W2, W3):
    seq = np.ascontiguousarray(np.asarray(seq, dtype=np.float32))
    consts = {
        "Wkv": np.ascontiguousarray(np.asarray(Wkv, np.float32)),
        "W0": np.ascontiguousarray(np.asarray(W0, np.float32)),
        "W1": np.ascontiguousarray(np.asarray(W1, np.float32)),
        "W2": np.ascontiguousarray(np.asarray(W2, np.float32)),
        "W3": np.ascontiguousarray(np.asarray(W3, np.float32)),
    }
    in_maps = []
    for c in range(NCORES):
        b, q = c // QUARTERS, c % QUARTERS
        own = seq[b, q * TOK_OWN : (q + 1) * TOK_OWN]
        prec = np.zeros((PREC_CHUNKS * CHUNK, D), np.float32)
        prec[: q * TOK_OWN] = seq[b, : q * TOK_OWN]
        in_maps.append({"seq_own": np.ascontiguousarray(own), "seq_prec": prec, **consts})
    return in_maps


def assemble(results):
    cum = np.empty((4, B, N, D, D), np.float32)
    nm = np.empty((4, B, D, D), np.float32)
    losses = np.empty((B * N,), np.float32)
    for c in range(NCORES):
        b, q = c // QUARTERS, c % QUARTERS
        r = results[c]
        cum[:, b, q * TOK_OWN : (q + 1) * TOK_OWN] = r["cum_part"]
        losses[b * N + q * TOK_OWN : b * N + (q + 1) * TOK_OWN] = r["loss_part"][0]
        if q == QUARTERS - 1:
            nm[:, b] = r["nm_part"]
    return cum, nm, losses


def _run(inputs, **spmd_kwargs):
    from concourse.bass_utils import run_bass_kernel_spmd

    nc = _get_program()
    in_maps = make_in_maps(
        inputs["seq"], inputs["Wkv"], inputs["W0"], inputs["W1"], inputs["W2"], inputs["W3"]
    )
    res = run_bass_kernel_spmd(nc, in_maps, list(range(NCORES)), **spmd_kwargs)
    return assemble(res.results), res


def kernel(**inputs):
    out, _ = _run(inputs)
    return out


# revision 7
# speedup vs baseline: 1.0338x; 1.0338x over previous
"""Trainium2 Bass kernel for nn_NeuralMemory (scatter_memory).

Computation (see reference): per-token MLP loss gradients w.r.t. W0..W3 are
rank-1 outer products gv_l[t] (x) av_l[t]; the output cum_grads is their
cumulative sum over the sequence, next_mem = W + cum[-1], plus per-token
losses.

Sharding: 8 cores, each owns (batch b = core//4, quarter q = core%4) a
256-token contiguous slice.  Cores recompute the fwd/bwd vectors of all
*preceding* tokens in their batch (zero-padded to a fixed 768) to build the
cross-chunk carry locally — fully SPMD, no collectives.

Per-core algorithm (all fp32):
  - fwd/bwd per 128-token chunk in "column" layout (features on partitions):
    kv projection, 4-layer MLP with silu, backprop to get gv_l / av_l, then
    PE-transpose to "row" layout (tokens on partitions).
  - carry: accumulated K=128 chunk-sum matmuls in one PSUM bank
    (4 layer slots of 128 fp32).
  - scan (the big part): for each token t of an own chunk, ONE matmul
    computes the full within-chunk prefix Gr[0:t+1].T @ Ar[0:t+1] into its
    own PSUM slot (K=t+1, N=128); the chunk carry is pre-injected into each
    slot by an identity matmul (N=512 per bank).  Whole-bank (128,512)
    copies evict 4 tokens at a time, alternating Vector/Scalar engines, into
    16-token SBUF staging tiles that are DMAed to DRAM (1 MiB per DMA,
    512 B descriptors).
"""

import numpy as np
from contextlib import ExitStack

import concourse.bass as bass
import concourse.bacc as bacc
import concourse.mybir as mybir
import concourse.tile as tile
from concourse.masks import make_identity

F32 = mybir.dt.float32
F16 = mybir.dt.float16
AF = mybir.ActivationFunctionType
OP = mybir.AluOpType

D = 128
B, N = 2, 1024
NCORES = 8
QUARTERS = 4
CHUNK = 128
TOK_OWN = N // QUARTERS              # 256
OWN_CHUNKS = TOK_OWN // CHUNK        # 2
PREC_CHUNKS = (QUARTERS - 1) * TOK_OWN // CHUNK  # 6
GB = 4                               # tokens per psum scan group (1 bank)
SG = 16                              # tokens per staging tile / DMA

# Use the hardware Silu / Derivative_silu activation LUTs (not modeled in
# CoreSim).  False = decompose via Sigmoid (validated in sim).
USE_LUT_SILU = False

# dtype of the scan operands (Gr/Ar rows, identity, carry replicas).  fp16
# gives single-pass PE matmuls + fast weight load (fp32 is hi/lo split = 2x).
SCAN_DT = F16


def build_program(own_chunks=OWN_CHUNKS, prec_chunks=PREC_CHUNKS):
    nc = bacc.Bacc("TRN2")
    n_own = own_chunks * CHUNK
    n_prec = prec_chunks * CHUNK

    seq_own = nc.dram_tensor("seq_own", [n_own, D], F32, kind="ExternalInput")
    seq_prec = nc.dram_tensor("seq_prec", [n_prec, D], F32, kind="ExternalInput")
    wkv_d = nc.dram_tensor("Wkv", [2 * D, D], F32, kind="ExternalInput")
    w_d = [nc.dram_tensor(f"W{l}", [D, D], F32, kind="ExternalInput") for l in range(4)]
    cum_out = nc.dram_tensor("cum_part", [4, n_own, D, D], F32, kind="ExternalOutput")
    loss_out = nc.dram_tensor("loss_part", [1, n_own], F32, kind="ExternalOutput")
    nm_out = nc.dram_tensor("nm_part", [4, D, D], F32, kind="ExternalOutput")

    with ExitStack() as ctx:
        tc = ctx.enter_context(tile.TileContext(nc))
        const = ctx.enter_context(tc.tile_pool(name="const", bufs=1))
        work = ctx.enter_context(tc.tile_pool(name="work", bufs=2))
        rows_own = ctx.enter_context(tc.tile_pool(name="rows_own", bufs=1))
        rows_prec = ctx.enter_context(tc.tile_pool(name="rows_prec", bufs=2))
        cq_pool = ctx.enter_context(tc.tile_pool(name="cq", bufs=2))
        stage_pool = ctx.enter_context(tc.tile_pool(name="stage", bufs=3))
        psum_work = ctx.enter_context(tc.tile_pool(name="psum_work", bufs=2, space="PSUM"))
        psum_carry = ctx.enter_context(tc.tile_pool(name="psum_carry", bufs=1, space="PSUM"))
        psum_scan = ctx.enter_context(tc.tile_pool(name="psum_scan", bufs=4, space="PSUM"))

        # ---- constants / weights ----
        ident = const.tile([D, D], F32, tag="ident")
        make_identity(nc, ident[:])
        ident_s = const.tile([D, D], SCAN_DT, tag="ident_s")
        make_identity(nc, ident_s[:])
        ones = const.tile([D, 1], F32, tag="ones")
        nc.vector.memset(ones[:], 1.0)

        w_raw = []   # Wk, Wv, W0..W3 as stored (row i on partition i)
        names = ["wk", "wv", "w0", "w1", "w2", "w3"]
        srcs = [wkv_d[0:D, :], wkv_d[D : 2 * D, :]] + [w[:] for w in w_d]
        for name, src in zip(names, srcs):
            t = const.tile([D, D], F32, tag=name)
            nc.gpsimd.dma_start(t[:], src)
            w_raw.append(t)
        w_t = []     # transposed copies (column k on partition k)
        for name, t in zip(names, w_raw):
            tp = psum_work.tile([D, D], F32, tag="work")
            nc.tensor.transpose(tp[:], t[:], ident[:])
            tt = const.tile([D, D], F32, tag=name + "T")
            nc.vector.tensor_copy(tt[:], tp[:])
            w_t.append(tt)
        wkT, wvT = w_t[0], w_t[1]
        wT = w_t[2:]          # fwd lhsT (W_l^T)
        wraw = w_raw[2:]      # bwd lhsT (W_l)

        loss_sb = const.tile([1, n_own], F32, tag="loss")

        # ---- fwd/bwd for one 128-token chunk ----
        def fwd_bwd(src_ap, rowpool, tag_sfx, loss_slice=None):
            x_r = work.tile([CHUNK, D], F32, tag="x_r")
            nc.gpsimd.dma_start(x_r[:], src_ap)
            xp = psum_work.tile([D, CHUNK], F32, tag="work")
            nc.tensor.transpose(xp[:], x_r[:], ident[:])
            x_c = work.tile([D, CHUNK], F32, tag="x_c")
            nc.scalar.copy(x_c[:], xp[:])

            kp = psum_work.tile([D, CHUNK], F32, tag="work")
            nc.tensor.matmul(kp[:], wkT[:], x_c[:])
            k_s = work.tile([D, CHUNK], F32, tag="k_s")
            nc.scalar.copy(k_s[:], kp[:])
            vp = psum_work.tile([D, CHUNK], F32, tag="work")
            nc.tensor.matmul(vp[:], wvT[:], x_c[:])
            v_s = work.tile([D, CHUNK], F32, tag="v_s")
            nc.scalar.mul(v_s[:], vp[:], 2.0 / D)

            acts = [k_s]
            dsil = []
            cur = k_s
            for l in range(3):
                zp = psum_work.tile([D, CHUNK], F32, tag="work")
                nc.tensor.matmul(zp[:], wT[l][:], cur[:])
                a = work.tile([D, CHUNK], F32, tag=f"a{l}")
                d = work.tile([D, CHUNK], F32, tag=f"d{l}")
                if USE_LUT_SILU:
                    nc.scalar.activation(a[:], zp[:], AF.Silu)
                    nc.scalar.activation(d[:], zp[:], AF.Derivative_silu)
                else:
                    s = work.tile([D, CHUNK], F32, tag=f"s{l}")
                    nc.scalar.activation(s[:], zp[:], AF.Sigmoid)
                    nc.vector.tensor_tensor(a[:], zp[:], s[:], OP.mult)
                    m = work.tile([D, CHUNK], F32, tag=f"m{l}")
                    nc.vector.tensor_tensor(m[:], a[:], s[:], OP.mult)
                    nc.vector.tensor_tensor(m[:], a[:], m[:], OP.subtract)
                    nc.vector.tensor_tensor(d[:], s[:], m[:], OP.add)
                acts.append(a)
                dsil.append(d)
                cur = a
            z4p = psum_work.tile([D, CHUNK], F32, tag="work")
            nc.tensor.matmul(z4p[:], wT[3][:], cur[:])
            e = work.tile([D, CHUNK], F32, tag="e")
            nc.vector.scalar_tensor_tensor(e[:], z4p[:], 2.0 / D, v_s[:], OP.mult, OP.subtract)

            if loss_slice is not None:
                sq = work.tile([D, CHUNK], F32, tag="sq")
                nc.scalar.activation(sq[:], e[:], AF.Square)
                lp = psum_work.tile([1, CHUNK], F32, tag="work")
                nc.tensor.matmul(lp[:], ones[:], sq[:])
                nc.scalar.mul(loss_slice, lp[:], D / 4.0)

            gvs = [None, None, None, e]
            for l in (3, 2, 1):
                up = psum_work.tile([D, CHUNK], F32, tag="work")
                nc.tensor.matmul(up[:], wraw[l][:], gvs[l][:])
                g = work.tile([D, CHUNK], F32, tag=f"gv{l - 1}")
                nc.vector.tensor_tensor(g[:], up[:], dsil[l - 1][:], OP.mult)
                gvs[l - 1] = g

            Gr, Ar = [], []
            for l in range(4):
                gp = psum_work.tile([CHUNK, D], F32, tag="work")
                nc.tensor.transpose(gp[:], gvs[l][:], ident[:])
                gr = rowpool.tile([CHUNK, D], SCAN_DT, tag=f"gr{l}{tag_sfx}")
                nc.scalar.copy(gr[:], gp[:])
                ap_ = psum_work.tile([CHUNK, D], F32, tag="work")
                nc.tensor.transpose(ap_[:], acts[l][:], ident[:])
                ar = rowpool.tile([CHUNK, D], SCAN_DT, tag=f"ar{l}{tag_sfx}")
                nc.scalar.copy(ar[:], ap_[:])
                Gr.append(gr)
                Ar.append(ar)
            return Gr, Ar

        # ---- phase 1: preceding chunks -> carry (4 layer slots in 1 bank) ----
        carry_p = psum_carry.tile([D, 4 * D], F32, tag="carry")
        for pc in range(prec_chunks):
            GrP, ArP = fwd_bwd(seq_prec[pc * CHUNK : (pc + 1) * CHUNK, :], rows_prec, "p")
            for l in range(4):
                nc.tensor.matmul(
                    carry_p[:, l * D : (l + 1) * D],
                    GrP[l][:],
                    ArP[l][:],
                    start=(pc == 0 and l == 0),
                    stop=(pc == prec_chunks - 1 and l == 3),
                )
        carry_sb = const.tile([D, 4 * D], F32, tag="carry_sb")
        nc.vector.tensor_copy(carry_sb[:], carry_p[:])

        # ---- phase 2: own chunks fwd/bwd (+losses), rows kept resident ----
        own_rows = []
        for oc in range(own_chunks):
            Gr, Ar = fwd_bwd(
                seq_own[oc * CHUNK : (oc + 1) * CHUNK, :],
                rows_own,
                f"o{oc}",
                loss_slice=loss_sb[0:1, oc * CHUNK : (oc + 1) * CHUNK],
            )
            own_rows.append((Gr, Ar))

        # ---- phase 2b: chunk-0 sums -> chunk-1 carry (PE, decoupled) ----
        carry_srcs = [carry_sb]
        if own_chunks > 1:
            s0_p = psum_carry.tile([D, 4 * D], F32, tag="carry")
            Gr0, Ar0 = own_rows[0]
            for l in range(4):
                nc.tensor.matmul(
                    s0_p[:, l * D : (l + 1) * D], Gr0[l][:], Ar0[l][:],
                    start=(l == 0), stop=(l == 3),
                )
            comb_sb = const.tile([D, 4 * D], F32, tag="comb_sb")
            nc.vector.tensor_tensor(comb_sb[:], s0_p[:], carry_sb[:], OP.add)
            carry_srcs.append(comb_sb)

        # ---- phase 3: scan ----
        last_stage = [None] * 4
        for oc in range(own_chunks):
            # per-layer carry replicated 4x: fp32 for fused DVE eviction
            # adds (even banks), fp16 for PE injection (odd banks)
            cqs, cqs16 = [], []
            for l in range(4):
                cq = cq_pool.tile([D, 4 * D], F32, tag=f"cq{l}")
                cq16 = cq_pool.tile([D, 4 * D], SCAN_DT, tag=f"cq16_{l}")
                src = carry_srcs[oc][:, l * D : (l + 1) * D]
                for r in range(4):
                    nc.gpsimd.tensor_copy(cq[:, r * D : (r + 1) * D], src)
                    nc.gpsimd.tensor_copy(cq16[:, r * D : (r + 1) * D], src)
                cqs.append(cq)
                cqs16.append(cq16)

            Gr, Ar = own_rows[oc]
            n_base = oc * CHUNK
            for l in range(4):
                for sg0 in range(0, CHUNK, SG):
                    stage = stage_pool.tile([D, SG * D], F32, tag=f"stage{l}")
                    for g0 in range(sg0, sg0 + SG, GB):
                        scan = psum_scan.tile([D, GB * D], F32, tag="scan")
                        inject = (g0 // GB) % 2 == 1
                        if inject:
                            nc.tensor.matmul(
                                scan[:], ident_s[:], cqs16[l][:],
                                start=True, stop=False,
                            )
                        for j in range(GB):
                            t = g0 + j
                            nc.tensor.matmul(
                                scan[:, j * D : (j + 1) * D],
                                Gr[l][0 : t + 1, :],
                                Ar[l][0 : t + 1, :],
                                start=(j == 0 and not inject),
                                stop=(j == GB - 1),
                            )
                        ssl = slice((g0 - sg0) * D, (g0 - sg0 + GB) * D)
                        if inject:
                            nc.scalar.copy(stage[:, ssl], scan[:])
                        else:
                            nc.vector.tensor_tensor(
                                stage[:, ssl], scan[:], cqs[l][:], OP.add
                            )
                    dma_eng = nc.sync if (l + sg0 // SG) % 2 == 0 else nc.scalar
                    dma_eng.dma_start(
                        cum_out[l, n_base + sg0 : n_base + sg0 + SG].rearrange("n i j -> i n j"),
                        stage[:].rearrange("p (n j) -> p n j", n=SG),
                    )
                    if sg0 + SG == CHUNK:
                        last_stage[l] = stage

        # ---- phase 4: next_mem + losses out ----
        for l in range(4):
            nm = work.tile([D, D], F32, tag=f"nm{l}")
            nc.vector.tensor_tensor(
                nm[:], last_stage[l][:, (SG - 1) * D : SG * D], wraw[l][:], OP.add
            )
            nc.sync.dma_start(nm_out[l], nm[:])
        nc.sync.dma_start(loss_out[:], loss_sb[:])

    nc.finalize()
    return nc


_PROG = None


def _get_program():
    global _PROG
    if _PROG is None:
        _PROG = build_program()
    return _PROG


def make_in_maps(seq, Wkv, W0, W1, W2, W3):
    seq = np.ascontiguousarray(np.asarray(seq, dtype=np.float32))
    consts = {
        "Wkv": np.ascontiguousarray(np.asarray(Wkv, np.float32)),
        "W0": np.ascontiguousarray(np.asarray(W0, np.float32)),
        "W1": np.ascontiguousarray(np.asarray(W1, np.float32)),
        "W2": np.ascontiguousarray(np.asarray(W2, np.float32)),
        "W3": np.ascontiguousarray(np.asarray(W3, np.float32)),
    }
    in_maps = []
    for c in range(NCORES):
        b, q = c // QUARTERS, c % QUARTERS
        own = seq[b, q * TOK_OWN : (q + 1) * TOK_OWN]
        prec = np.zeros((PREC_CHUNKS * CHUNK, D), np.float32)
        prec[: q * TOK_OWN] = seq[b, : q * TOK_OWN]
        in_maps.append({"seq_own": np.ascontiguousarray(own), "seq_prec": prec, **consts})
    return in_maps


def assemble(results):
    cum = np.empty((4, B, N, D, D), np.float32)
    nm = np.empty((4, B, D, D), np.float32)
    losses = np.empty((B * N,), np.float32)
    for c in range(NCORES):
        b, q = c // QUARTERS, c % QUARTERS
        r = results[c]
        cum[:, b, q * TOK_OWN : (q + 1) * TOK_OWN] = r["cum_part"]
        losses[b * N + q * TOK_OWN : b * N + (q + 1) * TOK_OWN] = r["loss_part"][0]
        if q == QUARTERS - 1:
            nm[:, b] = r["nm_part"]
    return cum, nm, losses


def _run(inputs, **spmd_kwargs):
    from concourse.bass_utils import run_bass_kernel_spmd

    nc = _get_program()
    in_maps = make_in_maps(
        inputs["seq"], inputs["Wkv"], inputs["W0"], inputs["W1"], inputs["W2"], inputs["W3"]
    )
    res = run_bass_kernel_spmd(nc, in_maps, list(range(NCORES)), **spmd_kwargs)
    return assemble(res.results), res


def kernel(**inputs):
    out, _ = _run(inputs)
    return out


# revision 9
# speedup vs baseline: 1.0464x; 1.0122x over previous
"""Trainium2 Bass kernel for nn_NeuralMemory (scatter_memory).

Computation (see reference): per-token MLP loss gradients w.r.t. W0..W3 are
rank-1 outer products gv_l[t] (x) av_l[t]; the output cum_grads is their
cumulative sum over the sequence, next_mem = W + cum[-1], plus per-token
losses.

Sharding: 8 cores, each owns (batch b = core//4, quarter q = core%4) a
256-token contiguous slice.  Cores recompute the fwd/bwd vectors of all
*preceding* tokens in their batch (zero-padded to a fixed 768) to build the
cross-chunk carry locally — fully SPMD, no collectives.

Per-core algorithm (all fp32):
  - fwd/bwd per 128-token chunk in "column" layout (features on partitions):
    kv projection, 4-layer MLP with silu, backprop to get gv_l / av_l, then
    PE-transpose to "row" layout (tokens on partitions).
  - carry: accumulated K=128 chunk-sum matmuls in one PSUM bank
    (4 layer slots of 128 fp32).
  - scan (the big part): for each token t of an own chunk, ONE matmul
    computes the full within-chunk prefix Gr[0:t+1].T @ Ar[0:t+1] into its
    own PSUM slot (K=t+1, N=128); the chunk carry is pre-injected into each
    slot by an identity matmul (N=512 per bank).  Whole-bank (128,512)
    copies evict 4 tokens at a time, alternating Vector/Scalar engines, into
    16-token SBUF staging tiles that are DMAed to DRAM (1 MiB per DMA,
    512 B descriptors).
"""

import numpy as np
from contextlib import ExitStack

import concourse.bass as bass
import concourse.bacc as bacc
import concourse.mybir as mybir
import concourse.tile as tile
from concourse.masks import make_identity

F32 = mybir.dt.float32
F16 = mybir.dt.float16
AF = mybir.ActivationFunctionType
OP = mybir.AluOpType

D = 128
B, N = 2, 1024
NCORES = 8
QUARTERS = 4
CHUNK = 128
TOK_OWN = N // QUARTERS              # 256
OWN_CHUNKS = TOK_OWN // CHUNK        # 2
PREC_CHUNKS = (QUARTERS - 1) * TOK_OWN // CHUNK  # 6
GB = 4                               # tokens per psum scan group (1 bank)
SG = 16                              # tokens per staging tile / DMA

# Use the hardware Silu / Derivative_silu activation LUTs (not modeled in
# CoreSim).  False = decompose via Sigmoid (validated in sim).
USE_LUT_SILU = False

# dtype of the scan operands (Gr/Ar rows, identity, carry replicas).  fp16
# gives single-pass PE matmuls + fast weight load (fp32 is hi/lo split = 2x).
SCAN_DT = F16


def build_program(own_chunks=OWN_CHUNKS, prec_chunks=PREC_CHUNKS):
    nc = bacc.Bacc("TRN2")
    n_own = own_chunks * CHUNK
    n_prec = prec_chunks * CHUNK

    seq_own = nc.dram_tensor("seq_own", [n_own, D], F32, kind="ExternalInput")
    seq_prec = nc.dram_tensor("seq_prec", [n_prec, D], F32, kind="ExternalInput")
    wkv_d = nc.dram_tensor("Wkv", [2 * D, D], F32, kind="ExternalInput")
    w_d = [nc.dram_tensor(f"W{l}", [D, D], F32, kind="ExternalInput") for l in range(4)]
    cum_out = nc.dram_tensor("cum_part", [4, n_own, D, D], F32, kind="ExternalOutput")
    loss_out = nc.dram_tensor("loss_part", [1, n_own], F32, kind="ExternalOutput")
    nm_out = nc.dram_tensor("nm_part", [4, D, D], F32, kind="ExternalOutput")
    warm_out = nc.dram_tensor("warm_part", [1, D], F32, kind="ExternalOutput")

    with ExitStack() as ctx:
        tc = ctx.enter_context(tile.TileContext(nc))
        const = ctx.enter_context(tc.tile_pool(name="const", bufs=1))
        work = ctx.enter_context(tc.tile_pool(name="work", bufs=2))
        rows_own = ctx.enter_context(tc.tile_pool(name="rows_own", bufs=1))
        rows_prec = ctx.enter_context(tc.tile_pool(name="rows_prec", bufs=2))
        cq_pool = ctx.enter_context(tc.tile_pool(name="cq", bufs=2))
        stage_pool = ctx.enter_context(tc.tile_pool(name="stage", bufs=4))
        psum_work = ctx.enter_context(tc.tile_pool(name="psum_work", bufs=2, space="PSUM"))
        psum_carry = ctx.enter_context(tc.tile_pool(name="psum_carry", bufs=1, space="PSUM"))
        psum_scan = ctx.enter_context(tc.tile_pool(name="psum_scan", bufs=4, space="PSUM"))

        # ---- constants / weights ----
        ident = const.tile([D, D], F32, tag="ident")
        make_identity(nc, ident[:])
        ident_s = const.tile([D, D], SCAN_DT, tag="ident_s")
        make_identity(nc, ident_s[:])

        # ~9us dense matmul burst to flip the PE HAM clock-gate to 2.4 GHz
        # before the real pipeline starts
        warm_sb = const.tile([1, D], F32, tag="warm_sb")
        for wi in range(10):
            wp = psum_scan.tile([D, 4 * D], F32, tag="scan")
            for wj in range(8):
                nc.tensor.matmul(
                    wp[:, 0:D], ident_s[:], ident_s[:],
                    start=(wj == 0), stop=(wj == 7),
                )
            if wi == 9:
                nc.vector.tensor_copy(warm_sb[:], wp[0:1, 0:D])
        nc.sync.dma_start(warm_out[:], warm_sb[:])
        ones = const.tile([D, 1], F32, tag="ones")
        nc.vector.memset(ones[:], 1.0)

        w_raw = []   # Wk, Wv, W0..W3 as stored (row i on partition i)
        names = ["wk", "wv", "w0", "w1", "w2", "w3"]
        srcs = [wkv_d[0:D, :], wkv_d[D : 2 * D, :]] + [w[:] for w in w_d]
        for name, src in zip(names, srcs):
            t = const.tile([D, D], F32, tag=name)
            nc.gpsimd.dma_start(t[:], src)
            w_raw.append(t)
        w_t = []     # transposed copies (column k on partition k)
        for name, t in zip(names, w_raw):
            tp = psum_work.tile([D, D], F32, tag="work")
            nc.tensor.transpose(tp[:], t[:], ident[:])
            tt = const.tile([D, D], F32, tag=name + "T")
            nc.vector.tensor_copy(tt[:], tp[:])
            w_t.append(tt)
        wkT, wvT = w_t[0], w_t[1]
        wT = w_t[2:]          # fwd lhsT (W_l^T)
        wraw = w_raw[2:]      # bwd lhsT (W_l)

        loss_sb = const.tile([1, n_own], F32, tag="loss")

        # ---- fwd/bwd for one 128-token chunk ----
        def fwd_bwd(src_ap, rowpool, tag_sfx, loss_slice=None):
            x_r = work.tile([CHUNK, D], F32, tag="x_r")
            nc.gpsimd.dma_start(x_r[:], src_ap)
            xp = psum_work.tile([D, CHUNK], F32, tag="work")
            nc.tensor.transpose(xp[:], x_r[:], ident[:])
            x_c = work.tile([D, CHUNK], F32, tag="x_c")
            nc.scalar.copy(x_c[:], xp[:])

            kp = psum_work.tile([D, CHUNK], F32, tag="work")
            nc.tensor.matmul(kp[:], wkT[:], x_c[:])
            k_s = work.tile([D, CHUNK], F32, tag="k_s")
            nc.scalar.copy(k_s[:], kp[:])
            vp = psum_work.tile([D, CHUNK], F32, tag="work")
            nc.tensor.matmul(vp[:], wvT[:], x_c[:])
            v_s = work.tile([D, CHUNK], F32, tag="v_s")
            nc.scalar.mul(v_s[:], vp[:], 2.0 / D)

            acts = [k_s]
            dsil = []
            cur = k_s
            for l in range(3):
                zp = psum_work.tile([D, CHUNK], F32, tag="work")
                nc.tensor.matmul(zp[:], wT[l][:], cur[:])
                a = work.tile([D, CHUNK], F32, tag=f"a{l}")
                d = work.tile([D, CHUNK], F32, tag=f"d{l}")
                if USE_LUT_SILU:
                    nc.scalar.activation(a[:], zp[:], AF.Silu)
                    nc.scalar.activation(d[:], zp[:], AF.Derivative_silu)
                else:
                    s = work.tile([D, CHUNK], F32, tag=f"s{l}")
                    nc.scalar.activation(s[:], zp[:], AF.Sigmoid)
                    nc.vector.tensor_tensor(a[:], zp[:], s[:], OP.mult)
                    m = work.tile([D, CHUNK], F32, tag=f"m{l}")
                    nc.vector.tensor_tensor(m[:], a[:], s[:], OP.mult)
                    nc.vector.tensor_tensor(m[:], a[:], m[:], OP.subtract)
                    nc.vector.tensor_tensor(d[:], s[:], m[:], OP.add)
                acts.append(a)
                dsil.append(d)
                cur = a
            z4p = psum_work.tile([D, CHUNK], F32, tag="work")
            nc.tensor.matmul(z4p[:], wT[3][:], cur[:])
            e = work.tile([D, CHUNK], F32, tag="e")
            nc.vector.scalar_tensor_tensor(e[:], z4p[:], 2.0 / D, v_s[:], OP.mult, OP.subtract)

            if loss_slice is not None:
                sq = work.tile([D, CHUNK], F32, tag="sq")
                nc.scalar.activation(sq[:], e[:], AF.Square)
                lp = psum_work.tile([1, CHUNK], F32, tag="work")
                nc.tensor.matmul(lp[:], ones[:], sq[:])
                nc.scalar.mul(loss_slice, lp[:], D / 4.0)

            gvs = [None, None, None, e]
            for l in (3, 2, 1):
                up = psum_work.tile([D, CHUNK], F32, tag="work")
                nc.tensor.matmul(up[:], wraw[l][:], gvs[l][:])
                g = work.tile([D, CHUNK], F32, tag=f"gv{l - 1}")
                nc.vector.tensor_tensor(g[:], up[:], dsil[l - 1][:], OP.mult)
                gvs[l - 1] = g

            Gr, Ar = [], []
            for l in range(4):
                gp = psum_work.tile([CHUNK, D], F32, tag="work")
                nc.tensor.transpose(gp[:], gvs[l][:], ident[:])
                gr = rowpool.tile([CHUNK, D], SCAN_DT, tag=f"gr{l}{tag_sfx}")
                nc.scalar.copy(gr[:], gp[:])
                ap_ = psum_work.tile([CHUNK, D], F32, tag="work")
                nc.tensor.transpose(ap_[:], acts[l][:], ident[:])
                ar = rowpool.tile([CHUNK, D], SCAN_DT, tag=f"ar{l}{tag_sfx}")
                nc.scalar.copy(ar[:], ap_[:])
                Gr.append(gr)
                Ar.append(ar)
            return Gr, Ar

        # ---- phase 1: preceding chunks -> carry (4 layer slots in 1 bank) ----
        carry_p = psum_carry.tile([D, 4 * D], F32, tag="carry")
        for pc in range(prec_chunks):
            GrP, ArP = fwd_bwd(seq_prec[pc * CHUNK : (pc + 1) * CHUNK, :], rows_prec, "p")
            for l in range(4):
                nc.tensor.matmul(
                    carry_p[:, l * D : (l + 1) * D],
                    GrP[l][:],
                    ArP[l][:],
                    start=(pc == 0 and l == 0),
                    stop=(pc == prec_chunks - 1 and l == 3),
                )
        carry_sb = const.tile([D, 4 * D], F32, tag="carry_sb")
        nc.vector.tensor_copy(carry_sb[:], carry_p[:])

        # ---- phase 2: own chunks fwd/bwd (+losses), rows kept resident ----
        own_rows = []
        for oc in range(own_chunks):
            Gr, Ar = fwd_bwd(
                seq_own[oc * CHUNK : (oc + 1) * CHUNK, :],
                rows_own,
                f"o{oc}",
                loss_slice=loss_sb[0:1, oc * CHUNK : (oc + 1) * CHUNK],
            )
            own_rows.append((Gr, Ar))

        # ---- phase 2b: chunk-0 sums -> chunk-1 carry (PE, decoupled) ----
        carry_srcs = [carry_sb]
        if own_chunks > 1:
            s0_p = psum_carry.tile([D, 4 * D], F32, tag="carry")
            Gr0, Ar0 = own_rows[0]
            for l in range(4):
                nc.tensor.matmul(
                    s0_p[:, l * D : (l + 1) * D], Gr0[l][:], Ar0[l][:],
                    start=(l == 0), stop=(l == 3),
                )
            comb_sb = const.tile([D, 4 * D], F32, tag="comb_sb")
            nc.vector.tensor_tensor(comb_sb[:], s0_p[:], carry_sb[:], OP.add)
            carry_srcs.append(comb_sb)

        # ---- phase 3: scan ----
        last_stage = [None] * 4
        for oc in range(own_chunks):
            # per-layer carry replicated 4x: fp32 for fused DVE eviction
            # adds (even banks), fp16 for PE injection (odd banks)
            cqs = []
            for l in range(4):
                cq = cq_pool.tile([D, 4 * D], F32, tag=f"cq{l}")
                src = carry_srcs[oc][:, l * D : (l + 1) * D]
                for r in range(4):
                    nc.gpsimd.tensor_copy(cq[:, r * D : (r + 1) * D], src)
                cqs.append(cq)

            Gr, Ar = own_rows[oc]
            n_base = oc * CHUNK
            for l in range(4):
                for sg0 in range(0, CHUNK, SG):
                    stage = stage_pool.tile([D, SG * D], F32, tag=f"stage{l}")
                    for g0 in range(sg0, sg0 + SG, GB):
                        scan = psum_scan.tile([D, GB * D], F32, tag="scan")
                        for j in range(GB):
                            t = g0 + j
                            nc.tensor.matmul(
                                scan[:, j * D : (j + 1) * D],
                                Gr[l][0 : t + 1, :],
                                Ar[l][0 : t + 1, :],
                                start=(j == 0),
                                stop=(j == GB - 1),
                            )
                        ssl = slice((g0 - sg0) * D, (g0 - sg0 + GB) * D)
                        if (g0 // GB) % 2 == 1:
                            nc.scalar.copy(stage[:, ssl], scan[:])
                            nc.gpsimd.tensor_tensor(
                                stage[:, ssl], stage[:, ssl], cqs[l][:], OP.add
                            )
                        else:
                            nc.vector.tensor_tensor(
                                stage[:, ssl], scan[:], cqs[l][:], OP.add
                            )
                    dma_eng = nc.sync if (l + sg0 // SG) % 2 == 0 else nc.scalar
                    dma_eng.dma_start(
                        cum_out[l, n_base + sg0 : n_base + sg0 + SG].rearrange("n i j -> i n j"),
                        stage[:].rearrange("p (n j) -> p n j", n=SG),
                    )
                    if sg0 + SG == CHUNK:
                        last_stage[l] = stage

        # ---- phase 4: next_mem + losses out ----
        for l in range(4):
            nm = work.tile([D, D], F32, tag=f"nm{l}")
            nc.vector.tensor_tensor(
                nm[:], last_stage[l][:, (SG - 1) * D : SG * D], wraw[l][:], OP.add
            )
            nc.sync.dma_start(nm_out[l], nm[:])
        nc.sync.dma_start(loss_out[:], loss_sb[:])

    nc.finalize()
    return nc


_PROG = None


def _get_program():
    global _PROG
    if _PROG is None:
        _PROG = build_program()
    return _PROG


def make_in_maps(seq, Wkv, W0, W1, W2, W3):
    seq = np.ascontiguousarray(np.asarray(seq, dtype=np.float32))
    consts = {
        "Wkv": np.ascontiguousarray(np.asarray(Wkv, np.float32)),
        "W0": np.ascontiguousarray(np.asarray(W0, np.float32)),
        "W1": np.ascontiguousarray(np.asarray(W1, np.float32)),
        "W2": np.ascontiguousarray(np.asarray(W2, np.float32)),
        "W3": np.ascontiguousarray(np.asarray(W3, np.float32)),
    }
    in_maps = []
    for c in range(NCORES):
        b, q = c // QUARTERS, c % QUARTERS
        own = seq[b, q * TOK_OWN : (q + 1) * TOK_OWN]
        prec = np.zeros((PREC_CHUNKS * CHUNK, D), np.float32)
        prec[: q * TOK_OWN] = seq[b, : q * TOK_OWN]
        in_maps.append({"seq_own": np.ascontiguousarray(own), "seq_prec": prec, **consts})
    return in_maps


def assemble(results):
    cum = np.empty((4, B, N, D, D), np.float32)
    nm = np.empty((4, B, D, D), np.float32)
    losses = np.empty((B * N,), np.float32)
    for c in range(NCORES):
        b, q = c // QUARTERS, c % QUARTERS
        r = results[c]
        cum[:, b, q * TOK_OWN : (q + 1) * TOK_OWN] = r["cum_part"]
        losses[b * N + q * TOK_OWN : b * N + (q + 1) * TOK_OWN] = r["loss_part"][0]
        if q == QUARTERS - 1:
            nm[:, b] = r["nm_part"]
    return cum, nm, losses


def _run(inputs, **spmd_kwargs):
    from concourse.bass_utils import run_bass_kernel_spmd

    nc = _get_program()
    in_maps = make_in_maps(
        inputs["seq"], inputs["Wkv"], inputs["W0"], inputs["W1"], inputs["W2"], inputs["W3"]
    )
    res = run_bass_kernel_spmd(nc, in_maps, list(range(NCORES)), **spmd_kwargs)
    return assemble(res.results), res


def kernel(**inputs):
    out, _ = _run(inputs)
    return out
